# revision 16
# baseline (speedup 1.0000x reference)
"""BitConv1d Trainium2 kernel.

Computes out[n,o,l] = conv1d(x, sign(w), pad=1) * mean(|w|) * scale, which is
mathematically identical to the reference

    x_scale = clip(mean(|x|, axis=(1,2)), 1e-5)
    out = conv1d(x / x_scale, sign(w), pad=1) * mean(|w|) * x_scale * scale

because conv is linear in x so the per-sample x_scale cancels exactly.

Sharding: data-parallel over batch N=16 across 8 cores (2 samples/core).

Device math: the PE array's native datapath is FP22 (e10m11).  float32
matmuls cost 4 passes; float32r costs 1 pass but rounds operands to
FP22.  Since sign(w) in {-1,0,1} is FP22-exact, we split
    hi  = round_fp22(x)      (DVE f32 -> f32r convert on write)
    lo  = x - hi             (<= 12 significant bits)
and accumulate matmul passes into fp32 PSUM:
  * hi pass: float32r, every product exact.
  * lo pass (lo_fp8=True): lo scaled by 2^12 and cast to fp8e4, pairs of
    input-channel chunks packed with perf_mode=DoubleRow (2 contraction
    elements per PE cell, half the matmul instructions).  Residual fp8
    quantization contributes ~2e-6 relative error.
  * lo pass (lo_fp8=False): float32r, near-exact (~1e-7 rel).
Outputs combine as (psum_hi + 2^-12 * psum_lo) * (mean|w| * scale).
"""

import numpy as np

# Problem geometry (hardcoded per contract).
N, C, L, KW = 16, 512, 4096, 3
NCORES = 8
NS = N // NCORES          # samples per core
P = 128                   # partitions
NTILE = 512               # moving free-dim per matmul
LO_FP8 = True             # fp8 DoubleRow lo-pass

_CACHE = {}


def _build_nc(ns=NS, c=C, length=L, kw=KW, repeat=1, lo_fp8=LO_FP8, nq=8):
    from contextlib import ExitStack
    from concourse import bacc, tile, mybir

    f32 = mybir.dt.float32
    f32r = mybir.dt.float32r
    fp8 = mybir.dt.float8e4
    Alu = mybir.AluOpType
    Act = mybir.ActivationFunctionType
    DR = mybir.MatmulPerfMode.DoubleRow

    pc_n = c // P             # input-channel chunks
    oc_n = c // P             # output-channel chunks
    pr_n = pc_n // 2          # fp8 chunk pairs
    hw = length // nq         # output columns per work item
    lt_n = hw // NTILE        # matmuls per psum bank row
    wcols = hw + 2            # with 1-col halo on each side
    wstride = (wcols + 15) // 16 * 16   # fp8 pair-plane stride, 16B aligned
    LO_SCALE = 2.0 ** 12

    nc = bacc.Bacc("TRN2", target_bir_lowering=False, debug=False)

    x_d = nc.dram_tensor("x", [ns, c, length], f32, kind="ExternalInput")
    w_d = nc.dram_tensor("wt", [kw, c, c], f32, kind="ExternalInput")
    s_d = nc.dram_tensor("scale", [1, 1], f32, kind="ExternalInput")
    o_d = nc.dram_tensor("out", [ns, c, length], f32, kind="ExternalOutput")

    with tile.TileContext(nc) as tc, ExitStack() as ctx:
        consts = ctx.enter_context(tc.tile_pool(name="consts", bufs=1))
        wst_p = ctx.enter_context(tc.tile_pool(name="wst", bufs=2))
        wsgn_p = ctx.enter_context(tc.tile_pool(name="wsgn", bufs=kw * pc_n))
        xs_p = ctx.enter_context(tc.tile_pool(name="xs", bufs=4))
        hi_p = ctx.enter_context(tc.tile_pool(name="hi", bufs=2 * pc_n))
        out_p = ctx.enter_context(tc.tile_pool(name="outs", bufs=4))
        psum_p = ctx.enter_context(
            tc.tile_pool(name="psum", bufs=2 * (2 if lo_fp8 else 1),
                         space="PSUM")
        )
        if lo_fp8:
            w8_p = ctx.enter_context(tc.tile_pool(name="w8", bufs=kw * pr_n))
            tmp_p = ctx.enter_context(tc.tile_pool(name="tmp", bufs=4))
            lo8_p = ctx.enter_context(
                tc.tile_pool(name="lo8", bufs=2 * pr_n))
        else:
            lo_p = ctx.enter_context(tc.tile_pool(name="lo", bufs=2 * pc_n))

        # ---------- setup: scale, sign(w), w_scale ----------
        sc = consts.tile([1, 1], f32, tag="sc")
        nc.sync.dma_start(sc[:, :], s_d[:, :])

        ones_col = consts.tile([P, 1], f32, tag="ones_col")
        nc.gpsimd.memset(ones_col[:], 1.0)
        ones_row = consts.tile([1, P], f32, tag="ones_row")
        nc.gpsimd.memset(ones_row[:], 1.0)

        partials = consts.tile([P, kw * pc_n], f32, tag="partials")
        wsgn = []
        w8 = {}
        if lo_fp8:
            for k in range(kw):
                for j in range(pr_n):
                    w8[k, j] = w8_p.tile([P, 2, c], fp8, tag="w8",
                                         name=f"w8_{k}_{j}")
        for k in range(kw):
            for pc in range(pc_n):
                wst = wst_p.tile([P, c], f32, tag="wst")
                nc.sync.dma_start(wst[:], w_d[k, pc * P:(pc + 1) * P, :])
                j = k * pc_n + pc
                nc.vector.tensor_reduce(
                    partials[:, j:j + 1], wst[:], mybir.AxisListType.X,
                    Alu.add, apply_absolute_value=True,
                )
                wt = wsgn_p.tile([P, c], f32r, tag="wsgn")
                nc.scalar.sign(wt[:], wst[:])
                wsgn.append(wt)
                if lo_fp8:
                    nc.scalar.sign(w8[k, pc // 2][:, pc % 2, :], wst[:])

        part1 = consts.tile([P, 1], f32, tag="part1")
        nc.vector.tensor_reduce(
            part1[:], partials[:], mybir.AxisListType.X, Alu.add
        )
        ps0 = psum_p.tile([P, hw], f32, tag="psum")
        nc.tensor.matmul(ps0[0:1, 0:1], part1[:], ones_col[:],
                         start=True, stop=True)
        tot = consts.tile([1, 1], f32, tag="tot")
        nc.vector.tensor_copy(tot[:], ps0[0:1, 0:1])
        c1 = consts.tile([1, 1], f32, tag="c1")
        nc.vector.tensor_tensor(c1[:], tot[:], sc[:], op=Alu.mult)
        nc.vector.tensor_scalar_mul(c1[:], c1[:], 1.0 / (c * c * kw))
        ps1 = psum_p.tile([P, hw], f32, tag="psum")
        nc.tensor.matmul(ps1[:, 0:1], ones_row[:], c1[:],
                         start=True, stop=True)
        cb = consts.tile([P, 1], f32, tag="cb")
        nc.vector.tensor_copy(cb[:], ps1[:, 0:1])
        if lo_fp8:
            cb12 = consts.tile([P, 1], f32, tag="cb12")
            nc.vector.tensor_scalar_mul(cb12[:], cb[:], 1.0 / LO_SCALE)

        # ---------- main loop ----------
        for s in [si for _ in range(repeat) for si in range(ns)]:
            for q in range(nq):
                his, los = [], []
                lo8 = {}
                if lo_fp8:
                    for j in range(pr_n):
                        lo8[j] = lo8_p.tile([P, 2, wstride], fp8, tag="lo8",
                                            name=f"lo8_{j}")
                for pc in range(pc_n):
                    xs = xs_p.tile([P, wcols], f32, tag="xs")
                    rows = slice(pc * P, (pc + 1) * P)
                    if q == 0:
                        nc.gpsimd.memset(xs[:, 0:1], 0.0)
                        nc.sync.dma_start(xs[:, 1:wcols],
                                          x_d[s, rows, 0:hw + 1])
                    elif q == nq - 1:
                        nc.gpsimd.memset(xs[:, wcols - 1:wcols], 0.0)
                        nc.sync.dma_start(xs[:, 0:wcols - 1],
                                          x_d[s, rows, q * hw - 1:length])
                    else:
                        nc.sync.dma_start(
                            xs[:, :],
                            x_d[s, rows, q * hw - 1:(q + 1) * hw + 1])
                    hi = hi_p.tile([P, wcols], f32r, tag="hi")
                    nc.vector.tensor_copy(hi[:], xs[:])
                    his.append(hi)
                    if lo_fp8:
                        tmp = tmp_p.tile([P, wcols], f32, tag="tmp")
                        nc.vector.tensor_tensor(tmp[:], xs[:], hi[:],
                                                op=Alu.subtract)
                        nc.vector.tensor_scalar_mul(
                            lo8[pc // 2][:, pc % 2, 0:wcols], tmp[:],
                            LO_SCALE)
                    else:
                        lo = lo_p.tile([P, wcols], f32r, tag="lo")
                        nc.vector.tensor_tensor(lo[:], xs[:], hi[:],
                                                op=Alu.subtract)
                        los.append(lo)

                for oc in range(oc_n):
                    ps_hi = psum_p.tile([P, hw], f32, tag="psum")
                    n_hi = pc_n * kw
                    hi_stop = lo_fp8  # close group here only in fp8 mode
                    j = 0
                    for pc in range(pc_n):
                        for k in range(kw):
                            lhsT = wsgn[k * pc_n + pc][:, oc * P:(oc + 1) * P]
                            for lt in range(lt_n):
                                nc.tensor.matmul(
                                    ps_hi[:, lt * NTILE:(lt + 1) * NTILE],
                                    lhsT,
                                    his[pc][:, lt * NTILE + k:
                                            lt * NTILE + k + NTILE],
                                    start=j == 0,
                                    stop=hi_stop and j == n_hi - 1,
                                )
                            j += 1
                    if not lo_fp8:
                        j = 0
                        for pc in range(pc_n):
                            for k in range(kw):
                                lhsT = wsgn[k * pc_n + pc][
                                    :, oc * P:(oc + 1) * P]
                                for lt in range(lt_n):
                                    nc.tensor.matmul(
                                        ps_hi[:, lt * NTILE:
                                              (lt + 1) * NTILE],
                                        lhsT,
                                        los[pc][:, lt * NTILE + k:
                                                lt * NTILE + k + NTILE],
                                        start=False,
                                        stop=j == n_hi - 1,
                                    )
                                j += 1
                        ot = out_p.tile([P, hw], f32, tag="outs")
                        nc.scalar.activation(ot[:], ps_hi[:], Act.Copy,
                                             scale=cb[:])
                        nc.sync.dma_start(
                            o_d[s, oc * P:(oc + 1) * P,
                                q * hw:(q + 1) * hw], ot[:])
                        continue

                    ps_lo = psum_p.tile([P, hw], f32, tag="psum")
                    n_lo = pr_n * kw
                    j = 0
                    for pr in range(pr_n):
                        for k in range(kw):
                            lhsT = w8[k, pr][:, :, oc * P:(oc + 1) * P]
                            for lt in range(lt_n):
                                nc.tensor.matmul(
                                    ps_lo[:, lt * NTILE:(lt + 1) * NTILE],
                                    lhsT,
                                    lo8[pr][:, :, lt * NTILE + k:
                                            lt * NTILE + k + NTILE],
                                    start=j == 0, stop=j == n_lo - 1,
                                    perf_mode=DR,
                                )
                            j += 1
                    t = out_p.tile([P, hw], f32, tag="outs")
                    nc.scalar.activation(t[:], ps_lo[:], Act.Copy,
                                         scale=cb12[:])
                    ot = out_p.tile([P, hw], f32, tag="outs")
                    nc.vector.scalar_tensor_tensor(
                        ot[:], ps_hi[:], cb[:], t[:],
                        op0=Alu.mult, op1=Alu.add)
                    nc.sync.dma_start(
                        o_d[s, oc * P:(oc + 1) * P, q * hw:(q + 1) * hw],
                        ot[:])

    nc.compile()
    return nc


def _get_nc(key=None):
    if key is None:
        key = (NS, C, L, KW)
    if key not in _CACHE:
        _CACHE[key] = _build_nc(*key)
    return _CACHE[key]


def _shard_inputs(x, weight, scale):
    x = np.ascontiguousarray(np.asarray(x, dtype=np.float32))
    weight = np.asarray(weight, dtype=np.float32)
    scale = np.asarray(scale, dtype=np.float32).reshape(1, 1)
    # [C_out, C_in, K] -> [K, C_in, C_out] so DMA reads are contiguous
    wt = np.ascontiguousarray(weight.transpose(2, 1, 0))
    return [
        {"x": x[i * NS:(i + 1) * NS], "wt": wt, "scale": scale}
        for i in range(NCORES)
    ]


def run_shards(in_maps, trace=False, **kw):
    from concourse.bass_utils import run_bass_kernel_spmd

    nc = _get_nc()
    return run_bass_kernel_spmd(nc, in_maps, list(range(NCORES)),
                                trace=trace, **kw)


def kernel(x, weight, scale):
    res = run_shards(_shard_inputs(x, weight, scale))
    return np.concatenate([r["out"] for r in res.results], axis=0)


# revision 17
# speedup vs baseline: 1.1097x; 1.1097x over previous
"""BitConv1d Trainium2 kernel.

Computes out[n,o,l] = conv1d(x, sign(w), pad=1) * mean(|w|) * scale, which is
mathematically identical to the reference

    x_scale = clip(mean(|x|, axis=(1,2)), 1e-5)
    out = conv1d(x / x_scale, sign(w), pad=1) * mean(|w|) * x_scale * scale

because conv is linear in x so the per-sample x_scale cancels exactly.

Sharding: data-parallel over batch N=16 across 8 cores (2 samples/core).

Device math: the PE array's native datapath is FP22 (e10m11).  float32
matmuls cost 4 passes; float32r costs 1 pass but rounds operands to
FP22.  Since sign(w) in {-1,0,1} is FP22-exact, we split
    hi  = round_fp22(x)      (DVE f32 -> f32r convert on write)
    lo  = x - hi             (<= 12 significant bits)
and accumulate matmul passes into fp32 PSUM:
  * hi pass: float32r, every product exact.
  * lo pass (lo_fp8=True): lo scaled by 2^12 and cast to fp8e4, pairs of
    input-channel chunks packed with perf_mode=DoubleRow (2 contraction
    elements per PE cell, half the matmul instructions).  Residual fp8
    quantization contributes ~2e-6 relative error.
  * lo pass (lo_fp8=False): float32r, near-exact (~1e-7 rel).
Outputs combine as (psum_hi + 2^-12 * psum_lo) * (mean|w| * scale).
"""

import numpy as np

# Problem geometry (hardcoded per contract).
N, C, L, KW = 16, 512, 4096, 3
NCORES = 8
NS = N // NCORES          # samples per core
P = 128                   # partitions
NTILE = 512               # moving free-dim per matmul
LO_FP8 = True             # fp8 DoubleRow lo-pass

_CACHE = {}


def _build_nc(ns=NS, c=C, length=L, kw=KW, repeat=1, lo_fp8=LO_FP8, nq=4):
    from contextlib import ExitStack
    from concourse import bacc, tile, mybir

    f32 = mybir.dt.float32
    f32r = mybir.dt.float32r
    fp8 = mybir.dt.float8e4
    Alu = mybir.AluOpType
    Act = mybir.ActivationFunctionType
    DR = mybir.MatmulPerfMode.DoubleRow

    pc_n = c // P             # input-channel chunks
    oc_n = c // P             # output-channel chunks
    pr_n = pc_n // 2          # fp8 chunk pairs
    hw = length // nq         # output columns per work item
    lt_n = hw // NTILE        # matmuls per psum bank row
    wcols = hw + 2            # with 1-col halo on each side
    wstride = (wcols + 15) // 16 * 16   # fp8 pair-plane stride, 16B aligned
    LO_SCALE = 2.0 ** 12

    nc = bacc.Bacc("TRN2", target_bir_lowering=False, debug=False)

    x_d = nc.dram_tensor("x", [ns, c, length], f32, kind="ExternalInput")
    w_d = nc.dram_tensor("wt", [kw, c, c], f32, kind="ExternalInput")
    s_d = nc.dram_tensor("scale", [1, 1], f32, kind="ExternalInput")
    o_d = nc.dram_tensor("out", [ns, c, length], f32, kind="ExternalOutput")

    with tile.TileContext(nc) as tc, ExitStack() as ctx:
        consts = ctx.enter_context(tc.tile_pool(name="consts", bufs=1))
        wst_p = ctx.enter_context(tc.tile_pool(name="wst", bufs=2))
        wsgn_p = ctx.enter_context(tc.tile_pool(name="wsgn", bufs=kw * pc_n))
        xs_p = ctx.enter_context(tc.tile_pool(name="xs", bufs=4))
        hi_p = ctx.enter_context(tc.tile_pool(name="hi", bufs=2 * pc_n))
        out_p = ctx.enter_context(tc.tile_pool(name="outs", bufs=4))
        psum_p = ctx.enter_context(
            tc.tile_pool(name="psum", bufs=2 * (2 if lo_fp8 else 1),
                         space="PSUM")
        )
        if lo_fp8:
            w8_p = ctx.enter_context(tc.tile_pool(name="w8", bufs=kw * pr_n))
            tmp_p = ctx.enter_context(tc.tile_pool(name="tmp", bufs=4))
            lo8_p = ctx.enter_context(
                tc.tile_pool(name="lo8", bufs=2 * pr_n))
        else:
            lo_p = ctx.enter_context(tc.tile_pool(name="lo", bufs=2 * pc_n))

        # ---------- setup: scale, sign(w), w_scale ----------
        sc = consts.tile([1, 1], f32, tag="sc")
        nc.sync.dma_start(sc[:, :], s_d[:, :])

        ones_col = consts.tile([P, 1], f32, tag="ones_col")
        nc.gpsimd.memset(ones_col[:], 1.0)
        ones_row = consts.tile([1, P], f32, tag="ones_row")
        nc.gpsimd.memset(ones_row[:], 1.0)

        partials = consts.tile([P, kw * pc_n], f32, tag="partials")
        wsgn = []
        w8 = {}
        if lo_fp8:
            for k in range(kw):
                for j in range(pr_n):
                    w8[k, j] = w8_p.tile([P, 2, c], fp8, tag="w8",
                                         name=f"w8_{k}_{j}")
        for k in range(kw):
            for pc in range(pc_n):
                wst = wst_p.tile([P, c], f32, tag="wst")
                nc.sync.dma_start(wst[:], w_d[k, pc * P:(pc + 1) * P, :])
                j = k * pc_n + pc
                nc.vector.tensor_reduce(
                    partials[:, j:j + 1], wst[:], mybir.AxisListType.X,
                    Alu.add, apply_absolute_value=True,
                )
                wt = wsgn_p.tile([P, c], f32r, tag="wsgn")
                nc.scalar.sign(wt[:], wst[:])
                wsgn.append(wt)
                if lo_fp8:
                    nc.scalar.sign(w8[k, pc // 2][:, pc % 2, :], wst[:])

        part1 = consts.tile([P, 1], f32, tag="part1")
        nc.vector.tensor_reduce(
            part1[:], partials[:], mybir.AxisListType.X, Alu.add
        )
        ps0 = psum_p.tile([P, hw], f32, tag="psum")
        nc.tensor.matmul(ps0[0:1, 0:1], part1[:], ones_col[:],
                         start=True, stop=True)
        tot = consts.tile([1, 1], f32, tag="tot")
        nc.vector.tensor_copy(tot[:], ps0[0:1, 0:1])
        c1 = consts.tile([1, 1], f32, tag="c1")
        nc.vector.tensor_tensor(c1[:], tot[:], sc[:], op=Alu.mult)
        nc.vector.tensor_scalar_mul(c1[:], c1[:], 1.0 / (c * c * kw))
        ps1 = psum_p.tile([P, hw], f32, tag="psum")
        nc.tensor.matmul(ps1[:, 0:1], ones_row[:], c1[:],
                         start=True, stop=True)
        cb = consts.tile([P, 1], f32, tag="cb")
        nc.vector.tensor_copy(cb[:], ps1[:, 0:1])
        if lo_fp8:
            cb12 = consts.tile([P, 1], f32, tag="cb12")
            nc.vector.tensor_scalar_mul(cb12[:], cb[:], 1.0 / LO_SCALE)

        # ---------- main loop ----------
        for s in [si for _ in range(repeat) for si in range(ns)]:
            for q in range(nq):
                his, los = [], []
                lo8 = {}
                if lo_fp8:
                    for j in range(pr_n):
                        lo8[j] = lo8_p.tile([P, 2, wstride], fp8, tag="lo8",
                                            name=f"lo8_{j}")
                for pc in range(pc_n):
                    xs = xs_p.tile([P, wcols], f32, tag="xs")
                    rows = slice(pc * P, (pc + 1) * P)
                    if q == 0:
                        nc.gpsimd.memset(xs[:, 0:1], 0.0)
                        nc.sync.dma_start(xs[:, 1:wcols],
                                          x_d[s, rows, 0:hw + 1])
                    elif q == nq - 1:
                        nc.gpsimd.memset(xs[:, wcols - 1:wcols], 0.0)
                        nc.sync.dma_start(xs[:, 0:wcols - 1],
                                          x_d[s, rows, q * hw - 1:length])
                    else:
                        nc.sync.dma_start(
                            xs[:, :],
                            x_d[s, rows, q * hw - 1:(q + 1) * hw + 1])
                    hi = hi_p.tile([P, wcols], f32r, tag="hi")
                    nc.vector.tensor_copy(hi[:], xs[:])
                    his.append(hi)
                    if lo_fp8:
                        tmp = tmp_p.tile([P, wcols], f32, tag="tmp")
                        nc.vector.tensor_tensor(tmp[:], xs[:], hi[:],
                                                op=Alu.subtract)
                        nc.vector.tensor_scalar_mul(
                            lo8[pc // 2][:, pc % 2, 0:wcols], tmp[:],
                            LO_SCALE)
                    else:
                        lo = lo_p.tile([P, wcols], f32r, tag="lo")
                        nc.vector.tensor_tensor(lo[:], xs[:], hi[:],
                                                op=Alu.subtract)
                        los.append(lo)

                for oc in range(oc_n):
                    ps_hi = psum_p.tile([P, hw], f32, tag="psum")
                    n_hi = pc_n * kw
                    hi_stop = lo_fp8  # close group here only in fp8 mode
                    j = 0
                    for pc in range(pc_n):
                        for k in range(kw):
                            lhsT = wsgn[k * pc_n + pc][:, oc * P:(oc + 1) * P]
                            for lt in range(lt_n):
                                nc.tensor.matmul(
                                    ps_hi[:, lt * NTILE:(lt + 1) * NTILE],
                                    lhsT,
                                    his[pc][:, lt * NTILE + k:
                                            lt * NTILE + k + NTILE],
                                    start=j == 0,
                                    stop=hi_stop and j == n_hi - 1,
                                )
                            j += 1
                    if not lo_fp8:
                        j = 0
                        for pc in range(pc_n):
                            for k in range(kw):
                                lhsT = wsgn[k * pc_n + pc][
                                    :, oc * P:(oc + 1) * P]
                                for lt in range(lt_n):
                                    nc.tensor.matmul(
                                        ps_hi[:, lt * NTILE:
                                              (lt + 1) * NTILE],
                                        lhsT,
                                        los[pc][:, lt * NTILE + k:
                                                lt * NTILE + k + NTILE],
                                        start=False,
                                        stop=j == n_hi - 1,
                                    )
                                j += 1
                        ot = out_p.tile([P, hw], f32, tag="outs")
                        nc.scalar.activation(ot[:], ps_hi[:], Act.Copy,
                                             scale=cb[:])
                        nc.sync.dma_start(
                            o_d[s, oc * P:(oc + 1) * P,
                                q * hw:(q + 1) * hw], ot[:])
                        continue

                    ps_lo = psum_p.tile([P, hw], f32, tag="psum")
                    n_lo = pr_n * kw
                    j = 0
                    for pr in range(pr_n):
                        for k in range(kw):
                            lhsT = w8[k, pr][:, :, oc * P:(oc + 1) * P]
                            for lt in range(lt_n):
                                nc.tensor.matmul(
                                    ps_lo[:, lt * NTILE:(lt + 1) * NTILE],
                                    lhsT,
                                    lo8[pr][:, :, lt * NTILE + k:
                                            lt * NTILE + k + NTILE],
                                    start=j == 0, stop=j == n_lo - 1,
                                    perf_mode=DR,
                                )
                            j += 1
                    t = out_p.tile([P, hw], f32, tag="outs")
                    nc.scalar.activation(t[:], ps_lo[:], Act.Copy,
                                         scale=cb12[:])
                    ot = out_p.tile([P, hw], f32, tag="outs")
                    nc.vector.scalar_tensor_tensor(
                        ot[:], ps_hi[:], cb[:], t[:],
                        op0=Alu.mult, op1=Alu.add)
                    nc.sync.dma_start(
                        o_d[s, oc * P:(oc + 1) * P, q * hw:(q + 1) * hw],
                        ot[:])

    nc.compile()
    return nc


def _get_nc(key=None):
    if key is None:
        key = (NS, C, L, KW)
    if key not in _CACHE:
        _CACHE[key] = _build_nc(*key)
    return _CACHE[key]


def _shard_inputs(x, weight, scale):
    x = np.ascontiguousarray(np.asarray(x, dtype=np.float32))
    weight = np.asarray(weight, dtype=np.float32)
    scale = np.asarray(scale, dtype=np.float32).reshape(1, 1)
    # [C_out, C_in, K] -> [K, C_in, C_out] so DMA reads are contiguous
    wt = np.ascontiguousarray(weight.transpose(2, 1, 0))
    return [
        {"x": x[i * NS:(i + 1) * NS], "wt": wt, "scale": scale}
        for i in range(NCORES)
    ]


def run_shards(in_maps, trace=False, **kw):
    from concourse.bass_utils import run_bass_kernel_spmd

    nc = _get_nc()
    return run_bass_kernel_spmd(nc, in_maps, list(range(NCORES)),
                                trace=trace, **kw)


def kernel(x, weight, scale):
    res = run_shards(_shard_inputs(x, weight, scale))
    return np.concatenate([r["out"] for r in res.results], axis=0)


# revision 27
# speedup vs baseline: 1.1943x; 1.0763x over previous
"""BitConv1d Trainium2 kernel.

Computes out[n,o,l] = conv1d(x, sign(w), pad=1) * mean(|w|) * scale, which is
mathematically identical to the reference

    x_scale = clip(mean(|x|, axis=(1,2)), 1e-5)
    out = conv1d(x / x_scale, sign(w), pad=1) * mean(|w|) * x_scale * scale

because conv is linear in x so the per-sample x_scale cancels exactly.

Sharding: data-parallel over batch N=16 across 8 cores (2 samples/core).

Device math: the PE array's native datapath is FP22 (e10m11).  float32
matmuls cost 4 passes; float32r costs 1 pass but rounds operands to
FP22.  Since sign(w) in {-1,0,1} is FP22-exact, we split
    hi  = round_fp22(x)      (DVE f32 -> f32r convert on write)
    lo  = x - hi             (<= 12 significant bits)
and accumulate matmul passes into fp32 PSUM:
  * hi pass: float32r, every product exact.
  * lo pass (lo_fp8=True): lo scaled by 2^12 and cast to fp8e4, pairs of
    input-channel chunks packed with perf_mode=DoubleRow (2 contraction
    elements per PE cell, half the matmul instructions).  Residual fp8
    quantization contributes ~2e-6 relative error.
  * lo pass (lo_fp8=False): float32r, near-exact (~1e-7 rel).
Outputs combine as (psum_hi + 2^-12 * psum_lo) * (mean|w| * scale).
"""

import numpy as np

# Problem geometry (hardcoded per contract).
N, C, L, KW = 16, 512, 4096, 3
NCORES = 8
NS = N // NCORES          # samples per core
P = 128                   # partitions
NTILE = 512               # moving free-dim per matmul
LO_FP8 = True             # fp8 DoubleRow lo-pass

_CACHE = {}


def _build_nc(ns=NS, c=C, length=L, kw=KW, repeat=1, lo_fp8=LO_FP8, nq=8):
    from contextlib import ExitStack
    from concourse import bacc, tile, mybir

    f32 = mybir.dt.float32
    f32r = mybir.dt.float32r
    fp8 = mybir.dt.float8e4
    Alu = mybir.AluOpType
    Act = mybir.ActivationFunctionType
    DR = mybir.MatmulPerfMode.DoubleRow

    pc_n = c // P             # input-channel chunks
    oc_n = c // P             # output-channel chunks
    pr_n = pc_n // 2          # fp8 chunk pairs
    hw = length // nq         # output columns per work item
    lt_n = hw // NTILE        # matmuls per psum bank row
    wcols = hw + 2            # with 1-col halo on each side
    wstride = (wcols + 15) // 16 * 16   # fp8 pair-plane stride, 16B aligned
    LO_SCALE = 2.0 ** 12

    nc = bacc.Bacc("TRN2", target_bir_lowering=False, debug=False)

    x_d = nc.dram_tensor("x", [ns, c, length], f32, kind="ExternalInput")
    w_d = nc.dram_tensor("wt", [kw, c, c], f32, kind="ExternalInput")
    s_d = nc.dram_tensor("scale", [1, 1], f32, kind="ExternalInput")
    o_d = nc.dram_tensor("out", [ns, c, length], f32, kind="ExternalOutput")

    with tile.TileContext(nc) as tc, ExitStack() as ctx:
        consts = ctx.enter_context(tc.tile_pool(name="consts", bufs=1))
        wst_p = ctx.enter_context(tc.tile_pool(name="wst", bufs=2))
        wsgn_p = ctx.enter_context(tc.tile_pool(name="wsgn", bufs=kw * pc_n))
        xs_p = ctx.enter_context(tc.tile_pool(name="xs", bufs=4))
        hi_p = ctx.enter_context(tc.tile_pool(name="hi", bufs=2 * pc_n))
        out_p = ctx.enter_context(tc.tile_pool(name="outs", bufs=4))
        psum_p = ctx.enter_context(
            tc.tile_pool(name="psum",
                         bufs=(8 if nq >= 8 else 4) if lo_fp8 else 2,
                         space="PSUM")
        )
        if lo_fp8:
            w8_p = ctx.enter_context(tc.tile_pool(name="w8", bufs=kw * pr_n))
            tmp_p = ctx.enter_context(tc.tile_pool(name="tmp", bufs=4))
            lo8_p = ctx.enter_context(
                tc.tile_pool(name="lo8", bufs=2 * pr_n))
        else:
            lo_p = ctx.enter_context(tc.tile_pool(name="lo", bufs=2 * pc_n))

        # x loads ride the SP (sync) DMA queues; weights ride the
        # Activation queues, so neither serializes the other.
        def prep_item(s, q):
            his = []
            lo8 = {}
            if lo_fp8:
                for j in range(pr_n):
                    lo8[j] = lo8_p.tile([P, 2, wstride], fp8, tag="lo8",
                                        name=f"lo8_{j}")
            los = []
            for pc in range(pc_n):
                xs = xs_p.tile([P, wcols], f32, tag="xs", name="xs")
                rows = slice(pc * P, (pc + 1) * P)
                if q == 0:
                    nc.gpsimd.memset(xs[:, 0:1], 0.0)
                    nc.sync.dma_start(xs[:, 1:wcols],
                                      x_d[s, rows, 0:hw + 1])
                elif q == nq - 1:
                    nc.gpsimd.memset(xs[:, wcols - 1:wcols], 0.0)
                    nc.sync.dma_start(xs[:, 0:wcols - 1],
                                      x_d[s, rows, q * hw - 1:length])
                else:
                    nc.sync.dma_start(
                        xs[:, :],
                        x_d[s, rows, q * hw - 1:(q + 1) * hw + 1])
                hi = hi_p.tile([P, wcols], f32r, tag="hi", name="hi")
                nc.vector.tensor_copy(hi[:], xs[:])
                his.append(hi)
                if lo_fp8:
                    tmp = tmp_p.tile([P, wcols], f32, tag="tmp", name="tmp")
                    nc.vector.tensor_tensor(tmp[:], xs[:], hi[:],
                                            op=Alu.subtract)
                    # scale+cast on ACT: keeps DVE under the PE rate
                    nc.scalar.activation(
                        lo8[pc // 2][:, pc % 2, 0:wcols], tmp[:],
                        Act.Copy, scale=float(LO_SCALE))
                else:
                    lo = lo_p.tile([P, wcols], f32r, tag="lo", name="lo")
                    nc.vector.tensor_tensor(lo[:], xs[:], hi[:],
                                            op=Alu.subtract)
                    los.append(lo)
            return his, los, lo8

        # ---------- setup: scale, sign(w), w_scale ----------
        sc = consts.tile([1, 1], f32, tag="sc")
        nc.scalar.dma_start(sc[:, :], s_d[:, :])

        partials = consts.tile([P, kw * pc_n], f32, tag="partials")
        wsgn = [None] * (kw * pc_n)
        w8 = {}
        if lo_fp8:
            for k in range(kw):
                for j in range(pr_n):
                    w8[k, j] = w8_p.tile([P, 2, c], fp8, tag="w8",
                                         name=f"w8_{k}_{j}")
        # pc-outer/k-inner matches the order the hi matmuls consume
        # stationaries; weight DMAs go via gpsimd queues to stay off the
        # x-DMA path.
        for pc in range(pc_n):
            for k in range(kw):
                wst = wst_p.tile([P, c], f32, tag="wst")
                nc.scalar.dma_start(wst[:], w_d[k, pc * P:(pc + 1) * P, :])
                j = k * pc_n + pc
                nc.vector.tensor_reduce(
                    partials[:, j:j + 1], wst[:], mybir.AxisListType.X,
                    Alu.add, apply_absolute_value=True,
                )
                wt = wsgn_p.tile([P, c], f32r, tag="wsgn")
                nc.scalar.sign(wt[:], wst[:])
                wsgn[k * pc_n + pc] = wt
                if lo_fp8:
                    nc.scalar.sign(w8[k, pc // 2][:, pc % 2, :], wst[:])

        # Partition reduce + broadcast on GpSimd: keeps the w_scale
        # scalar chain off the PE's in-order instruction stream, which
        # otherwise stalls every main matmul behind it (~30us).
        from concourse import bass_isa
        part1 = consts.tile([P, 1], f32, tag="part1")
        nc.vector.tensor_reduce(
            part1[:], partials[:], mybir.AxisListType.X, Alu.add
        )
        tot_b = consts.tile([P, 1], f32, tag="tot_b")
        nc.gpsimd.partition_all_reduce(tot_b[:], part1[:], P,
                                       bass_isa.ReduceOp.add)
        sc_b = consts.tile([P, 1], f32, tag="sc_b")
        nc.gpsimd.partition_broadcast(sc_b[:], sc[:])
        cb = consts.tile([P, 1], f32, tag="cb")
        nc.vector.scalar_tensor_tensor(
            cb[:], tot_b[:], 1.0 / (c * c * kw), sc_b[:],
            op0=Alu.mult, op1=Alu.mult)
        if lo_fp8:
            cb12 = consts.tile([P, 1], f32, tag="cb12")
            nc.vector.tensor_scalar_mul(cb12[:], cb[:], 1.0 / LO_SCALE)

        # ---------- main loop ----------
        for s in [si for _ in range(repeat) for si in range(ns)]:
            for q in range(nq):
                his, los, lo8 = prep_item(s, q)

                for oc in range(oc_n):
                    ps_hi = psum_p.tile([P, hw], f32, tag="psum")
                    n_hi = pc_n * kw
                    hi_stop = lo_fp8  # close group here only in fp8 mode
                    j = 0
                    for pc in range(pc_n):
                        for k in range(kw):
                            lhsT = wsgn[k * pc_n + pc][:, oc * P:(oc + 1) * P]
                            for lt in range(lt_n):
                                nc.tensor.matmul(
                                    ps_hi[:, lt * NTILE:(lt + 1) * NTILE],
                                    lhsT,
                                    his[pc][:, lt * NTILE + k:
                                            lt * NTILE + k + NTILE],
                                    start=j == 0,
                                    stop=hi_stop and j == n_hi - 1,
                                )
                            j += 1
                    if not lo_fp8:
                        j = 0
                        for pc in range(pc_n):
                            for k in range(kw):
                                lhsT = wsgn[k * pc_n + pc][
                                    :, oc * P:(oc + 1) * P]
                                for lt in range(lt_n):
                                    nc.tensor.matmul(
                                        ps_hi[:, lt * NTILE:
                                              (lt + 1) * NTILE],
                                        lhsT,
                                        los[pc][:, lt * NTILE + k:
                                                lt * NTILE + k + NTILE],
                                        start=False,
                                        stop=j == n_hi - 1,
                                    )
                                j += 1
                        ot = out_p.tile([P, hw], f32, tag="outs")
                        nc.scalar.activation(ot[:], ps_hi[:], Act.Copy,
                                             scale=cb[:])
                        nc.sync.dma_start(
                            o_d[s, oc * P:(oc + 1) * P,
                                q * hw:(q + 1) * hw], ot[:])
                        continue

                    ps_lo = psum_p.tile([P, hw], f32, tag="psum")
                    n_lo = pr_n * kw
                    j = 0
                    for pr in range(pr_n):
                        for k in range(kw):
                            lhsT = w8[k, pr][:, :, oc * P:(oc + 1) * P]
                            for lt in range(lt_n):
                                nc.tensor.matmul(
                                    ps_lo[:, lt * NTILE:(lt + 1) * NTILE],
                                    lhsT,
                                    lo8[pr][:, :, lt * NTILE + k:
                                            lt * NTILE + k + NTILE],
                                    start=j == 0, stop=j == n_lo - 1,
                                    perf_mode=DR,
                                )
                            j += 1
                    t = out_p.tile([P, hw], f32, tag="outs")
                    nc.scalar.activation(t[:], ps_lo[:], Act.Copy,
                                         scale=cb12[:])
                    ot = out_p.tile([P, hw], f32, tag="outs")
                    nc.vector.scalar_tensor_tensor(
                        ot[:], ps_hi[:], cb[:], t[:],
                        op0=Alu.mult, op1=Alu.add)
                    nc.sync.dma_start(
                        o_d[s, oc * P:(oc + 1) * P, q * hw:(q + 1) * hw],
                        ot[:])

    nc.compile()
    return nc


def _get_nc(key=None):
    if key is None:
        key = (NS, C, L, KW)
    if key not in _CACHE:
        _CACHE[key] = _build_nc(*key)
    return _CACHE[key]


def _shard_inputs(x, weight, scale):
    x = np.ascontiguousarray(np.asarray(x, dtype=np.float32))
    weight = np.asarray(weight, dtype=np.float32)
    scale = np.asarray(scale, dtype=np.float32).reshape(1, 1)
    # [C_out, C_in, K] -> [K, C_in, C_out] so DMA reads are contiguous
    wt = np.ascontiguousarray(weight.transpose(2, 1, 0))
    return [
        {"x": x[i * NS:(i + 1) * NS], "wt": wt, "scale": scale}
        for i in range(NCORES)
    ]


def run_shards(in_maps, trace=False, **kw):
    from concourse.bass_utils import run_bass_kernel_spmd

    nc = _get_nc()
    return run_bass_kernel_spmd(nc, in_maps, list(range(NCORES)),
                                trace=trace, **kw)


def kernel(x, weight, scale):
    res = run_shards(_shard_inputs(x, weight, scale))
    return np.concatenate([r["out"] for r in res.results], axis=0)
